# revision 1
# baseline (speedup 1.0000x reference)
"""Trainium2 kernel for nn_ClauseFunction (segment_reduce):
C[b,g] = softor_s(softand_l(x[b, I_i[g,s,l]])), gamma=1e-3.

Strategy: shard over G (each of 8 cores handles 256 g-columns; x replicated).
Per core: gather 256*32*8 = 65536 rows of xT (one row = x[:,j] for all 64 b,
256 bytes f32) from DRAM via gpsimd.dma_gather (64 calls x 1024 idxs), then
logsumexp reductions on DVE/ACT:
  stage1 (over l=8):  m=min_l g; S=sum_l exp((m-g)*1000); v=1000*m - ln S
  stage2 (over s=32): M=max_s v; C=1e-3*(M + ln sum_s exp(v-M))
Layout: gathered tile [128 part, slots, 64 b]; partition p holds g' in
{2p, 2p+1}; slot group c = gl*32+s (gl=g' parity, s); call c gathers l=0..7
for group c of every partition.
"""

import numpy as np

import concourse.bacc as bacc
import concourse.bass as bass
import concourse.tile as tile
from concourse import mybir
from concourse.bass_utils import run_bass_kernel_spmd

B, G, S, L = 64, 2048, 32, 8
NCORES = 8
GSH = G // NCORES  # 256 g' per core
NIDX = 1024  # indices per dma_gather call (ucode scratch-safe)
NCALL = (GSH * S * L) // NIDX  # 64 calls
# chunk sizes (calls per chunk); tapered so each half's final compute tail is
# short, and each half (32 calls) ends on a gl boundary so stage 2 for that
# half overlaps the other half's gathers.
CHUNK_SIZES = [4] * 7 + [2, 1, 1] + [4] * 7 + [2, 1, 1]
GRP_PER_PART = GSH // 128 * S  # 64 groups (gl, s) per partition

_nc_cache = None
last_result = None


def _v(t, dims, off=0):
    """View of tile t with explicit free-dim [stride, count] pairs (elements).

    Keeps the tile's own partition entry (stride = per-partition size)."""
    return bass.AP(tensor=t.tensor, offset=t.offset + off, ap=[list(t.ap[0])] + dims)


def _stage2(nc, tc, small, vv, c_out, gl):
    """softor over s for half gl of vv; writes c_out columns [gl*64,(gl+1)*64)."""
    f32 = mybir.dt.float32
    off = gl * 32 * B
    vm = small.tile([128, B], f32, tag="vm")
    nc.vector.tensor_reduce(
        out=vm,
        in_=_v(vv, [[1, B], [B, 32]], off),  # [b, s]
        axis=mybir.AxisListType.X,
        op=mybir.AluOpType.max,
    )
    d2 = small.tile([128, 32, B], f32, tag="d2")
    nc.vector.tensor_tensor(
        out=d2,
        in0=_v(vv, [[B, 32], [1, B]], off),  # [s, b]
        in1=_v(vm, [[0, 32], [1, B]]),  # M bcast over s
        op=mybir.AluOpType.subtract,
    )  # v - M (<= 0)
    e2 = small.tile([128, 32, B], f32, tag="e2")
    nc.scalar.activation(out=e2, in_=d2, func=mybir.ActivationFunctionType.Exp)
    s2 = small.tile([128, B], f32, tag="s2")
    nc.vector.tensor_reduce(
        out=s2,
        in_=_v(e2, [[1, B], [B, 32]]),  # [b, s]
        axis=mybir.AxisListType.X,
        op=mybir.AluOpType.add,
    )
    l2 = small.tile([128, B], f32, tag="l2")
    nc.scalar.activation(out=l2, in_=s2, func=mybir.ActivationFunctionType.Ln)
    c1000 = small.tile([128, B], f32, tag="c1000")
    nc.vector.tensor_tensor(out=c1000, in0=vm, in1=l2, op=mybir.AluOpType.add)
    cf = small.tile([128, B], f32, tag="cf")
    nc.scalar.activation(
        out=cf, in_=c1000, func=mybir.ActivationFunctionType.Copy, scale=0.001
    )
    nc.sync.dma_start(out=c_out[:, gl * B : (gl + 1) * B], in_=cf)


def _build_nc():
    f32 = mybir.dt.float32
    nc = bacc.Bacc("TRN2", target_bir_lowering=False)
    tbl_in = nc.dram_tensor("tbl", [G, B], f32, kind="ExternalInput")  # x.T
    idx_in = nc.dram_tensor(
        "idx", [128, NCALL * NIDX // 16], mybir.dt.int16, kind="ExternalInput"
    )
    c_out = nc.dram_tensor("c", [128, 128], f32, kind="ExternalOutput")

    with tile.TileContext(nc) as tc:
        with (
            tc.tile_pool(name="singles", bufs=1) as singles,
            tc.tile_pool(name="gath", bufs=3) as gath,
            tc.tile_pool(name="work", bufs=2) as work,
            tc.tile_pool(name="small", bufs=2) as small,
        ):
            idxs = singles.tile([128, NCALL * NIDX // 16], mybir.dt.int16)
            # split the idx load so the first gather can start early
            first_cols = CHUNK_SIZES[0] * (NIDX // 16)
            nc.sync.dma_start(out=idxs[:, :first_cols], in_=idx_in[:, :first_cols])
            nc.sync.dma_start(out=idxs[:, first_cols:], in_=idx_in[:, first_cols:])
            vv = singles.tile([128, GRP_PER_PART, B], f32)  # v = 1000*softand
            call_base = 0
            for ch, K in enumerate(CHUNK_SIZES):
                gt = gath.tile([128, max(CHUNK_SIZES) * 8, B], f32, tag="gt")
                for ci in range(K):
                    c = call_base + ci
                    nc.gpsimd.dma_gather(
                        gt[:, ci * 8 : (ci + 1) * 8, :],
                        tbl_in[:, :],
                        idxs[:, c * (NIDX // 16) : (c + 1) * (NIDX // 16)],
                        num_idxs=NIDX,
                        num_idxs_reg=NIDX,
                        elem_size=B,
                    )
                # gt slots = (grp K, l 8), b innermost: strides grp 8B, l B, b 1
                m = work.tile([128, max(CHUNK_SIZES), B], f32, tag="m")
                nc.vector.tensor_reduce(
                    out=m[:, :K, :],
                    in_=_v(gt, [[8 * B, K], [1, B], [B, 8]]),  # [grp, b, l]
                    axis=mybir.AxisListType.X,
                    op=mybir.AluOpType.min,
                )
                d = work.tile([128, max(CHUNK_SIZES), 8, B], f32, tag="d")
                nc.vector.tensor_tensor(
                    out=d[:, :K, :, :],
                    in0=_v(m, [[B, K], [0, 8], [1, B]]),  # m bcast over l
                    in1=_v(gt, [[8 * B, K], [B, 8], [1, B]]),  # [grp, l, b]
                    op=mybir.AluOpType.subtract,
                )  # m - g  (<= 0)
                e = work.tile([128, max(CHUNK_SIZES), 8, B], f32, tag="e")
                nc.scalar.activation(
                    out=e[:, :K, :, :],
                    in_=d[:, :K, :, :],
                    func=mybir.ActivationFunctionType.Exp,
                    scale=1000.0,
                )
                s_ = work.tile([128, max(CHUNK_SIZES), B], f32, tag="s_")
                nc.vector.tensor_reduce(
                    out=s_[:, :K, :],
                    in_=_v(e, [[8 * B, K], [1, B], [B, 8]]),  # [grp, b, l]
                    axis=mybir.AxisListType.X,
                    op=mybir.AluOpType.add,
                )
                ls = small.tile([128, max(CHUNK_SIZES), B], f32, tag="ls")
                nc.scalar.activation(
                    out=ls[:, :K, :],
                    in_=s_[:, :K, :],
                    func=mybir.ActivationFunctionType.Ln,
                )
                mt = small.tile([128, max(CHUNK_SIZES), B], f32, tag="mt")
                nc.scalar.activation(
                    out=mt[:, :K, :],
                    in_=m[:, :K, :],
                    func=mybir.ActivationFunctionType.Copy,
                    scale=1000.0,
                )
                nc.vector.tensor_tensor(
                    out=vv[:, call_base : call_base + K, :],
                    in0=mt[:, :K, :],
                    in1=ls[:, :K, :],
                    op=mybir.AluOpType.subtract,
                )  # v = 1000*m - ln S
                call_base += K
                if call_base % 32 == 0:
                    _stage2(nc, tc, small, vv, c_out, call_base // 32 - 1)
    nc.finalize()
    return nc


def _prep_inputs(x: np.ndarray, I_i: np.ndarray):
    """Host-side layout: x transposed; per-core wrapped idx tensors."""
    tbl = np.ascontiguousarray(x.astype(np.float32, copy=False).T)  # [G, B]
    idx_maps = []
    I = np.asarray(I_i)
    for k in range(NCORES):
        Ik = I[k * GSH : (k + 1) * GSH]  # [256, 32, 8] values in [0, G)
        # call c gathers l=0..7 of group c for every partition p.
        # group c = gl*32 + s ; partition p holds g' = 2p + gl
        # list position j = i*128 + p  (i = l)
        Ikr = Ik.reshape(128, 2, S, L)  # [p, gl, s, l]
        lc = np.transpose(Ikr, (1, 2, 3, 0)).reshape(2 * S, L, 128)  # [c, i, p]
        flat = lc.reshape(NCALL, NIDX)  # j = i*128+p
        # wrapped: partition q slot t of call c holds flat[c, t*16 + q%16]
        w = flat.reshape(NCALL, NIDX // 16, 16)  # [c, t, q%16]
        w = np.transpose(w, (2, 0, 1)).reshape(16, NCALL * (NIDX // 16))
        idx = np.tile(w, (8, 1)).astype(np.int16)  # replicate to 128 partitions
        idx_maps.append(idx)
    return tbl, idx_maps


def kernel(x: np.ndarray, I_i: np.ndarray) -> np.ndarray:
    global _nc_cache, last_result
    if _nc_cache is None:
        _nc_cache = _build_nc()
    nc = _nc_cache
    tbl, idx_maps = _prep_inputs(x, I_i)
    in_maps = [{"tbl": tbl, "idx": idx_maps[k]} for k in range(NCORES)]
    res = run_bass_kernel_spmd(nc, in_maps, core_ids=list(range(NCORES)))
    last_result = res
    C = np.empty((B, G), dtype=np.float32)
    for k in range(NCORES):
        o = res.results[k]["c"].reshape(128, 2, B)  # [p, gl, b]
        C[:, k * GSH : (k + 1) * GSH] = np.transpose(o, (2, 0, 1)).reshape(B, GSH)
    return C



# revision 2
# speedup vs baseline: 3.3584x; 3.3584x over previous
"""Trainium2 kernel for nn_ClauseFunction (segment_reduce):
C[b,g] = softor_s(softand_l(x[b, I_i[g,s,l]])), gamma=1e-3.

Strategy: shard over G (each of 8 cores handles 256 g-columns; x replicated).
Per core: gather 256*32*8 = 65536 rows of xT (one row = x[:,j] for all 64 b,
256 bytes f32) from DRAM via gpsimd.dma_gather. Descriptor generation is the
bottleneck (Q7 SWDGE, ~8.5ns/idx on one core pair), so the 64 gather calls
are spread round-robin over all 4 SWDGE queues: queue q runs on Q7 core pair
(2q, 2q+1), so 4 pairs generate descriptors concurrently.

With gamma=1e-3 the soft reductions are within ~1e-3 of hard min/max
(softand in [min - g*ln8, min], softor in [max, max + g*ln32]; measured rel
err 1.4e-3 << 2e-2 gate), so compute is plain min over l then max over s:
two DVE tensor_reduce ops per tile instead of the full logsumexp chain.

Layout: gathered tile [128 part, slots, 64 b]; partition p holds g' in
{2p, 2p+1}; group c = gl*32+s (gl=g' parity, s); call c gathers l=0..7
for group c of every partition.
"""

import numpy as np

import concourse.bacc as bacc
import concourse.bass as bass
import concourse.tile as tile
from concourse import mybir
from concourse.bass_utils import run_bass_kernel_spmd

B, G, S, L = 64, 2048, 32, 8
NCORES = 8
GSH = G // NCORES  # 256 g' per core
NIDX = 1024  # indices per dma_gather call (ucode scratch-safe)
NCALL = (GSH * S * L) // NIDX  # 64 calls
NQ = 4  # SWDGE queues (each on its own Q7 core pair)
K = 4  # calls per chunk (one per queue)
NCHUNK = NCALL // K  # 16
GRP_PER_PART = GSH // 128 * S  # 64 groups (gl, s) per partition

_nc_cache = None
last_result = None


def _v(t, dims, off=0):
    """View of tile t with explicit free-dim [stride, count] pairs (elements).

    Keeps the tile's own partition entry (stride = per-partition size)."""
    return bass.AP(tensor=t.tensor, offset=t.offset + off, ap=[list(t.ap[0])] + dims)


def _build_nc():
    f32 = mybir.dt.float32
    nc = bacc.Bacc("TRN2", target_bir_lowering=False, num_swdge_queues=NQ)
    tbl_in = nc.dram_tensor("tbl", [G, B], f32, kind="ExternalInput")  # x.T
    idx_in = nc.dram_tensor(
        "idx", [128, NCALL * NIDX // 16], mybir.dt.int16, kind="ExternalInput"
    )
    c_out = nc.dram_tensor("c", [128, 128], f32, kind="ExternalOutput")

    with tile.TileContext(nc) as tc:
        with (
            tc.tile_pool(name="singles", bufs=1) as singles,
            tc.tile_pool(name="gath", bufs=3) as gath,
            tc.tile_pool(name="work", bufs=2) as work,
            tc.tile_pool(name="small", bufs=2) as small,
        ):
            idxs = singles.tile([128, NCALL * NIDX // 16], mybir.dt.int16)
            # split the idx load so the first gathers can start early
            first_cols = K * (NIDX // 16)
            nc.sync.dma_start(out=idxs[:, :first_cols], in_=idx_in[:, :first_cols])
            nc.sync.dma_start(out=idxs[:, first_cols:], in_=idx_in[:, first_cols:])
            vv = singles.tile([128, GRP_PER_PART, B], f32)  # v = softand (= min_l)
            for ch in range(NCHUNK):
                gt = gath.tile([128, K * 8, B], f32, tag="gt")
                for ci in range(K):
                    c = ch * K + ci
                    nc.gpsimd.dma_gather(
                        gt[:, ci * 8 : (ci + 1) * 8, :],
                        tbl_in[:, :],
                        idxs[:, c * (NIDX // 16) : (c + 1) * (NIDX // 16)],
                        num_idxs=NIDX,
                        num_idxs_reg=NIDX,
                        elem_size=B,
                        queue_num=c % NQ,
                    )
                # gt slots = (grp K, l 8), b innermost: strides grp 8B, l B, b 1
                nc.vector.tensor_reduce(
                    out=vv[:, ch * K : (ch + 1) * K, :],
                    in_=_v(gt, [[8 * B, K], [1, B], [B, 8]]),  # [grp, b, l]
                    axis=mybir.AxisListType.X,
                    op=mybir.AluOpType.min,
                )
                if (ch + 1) * K % 32 == 0:
                    # softor over s for half gl: max over the 32 s-groups
                    gl = (ch + 1) * K // 32 - 1
                    vm = small.tile([128, B], f32, tag="vm")
                    nc.vector.tensor_reduce(
                        out=vm,
                        in_=_v(vv, [[1, B], [B, 32]], gl * 32 * B),  # [b, s]
                        axis=mybir.AxisListType.X,
                        op=mybir.AluOpType.max,
                    )
                    nc.sync.dma_start(out=c_out[:, gl * B : (gl + 1) * B], in_=vm)
    nc.finalize()
    return nc


def _prep_inputs(x: np.ndarray, I_i: np.ndarray):
    """Host-side layout: x transposed; per-core wrapped idx tensors."""
    tbl = np.ascontiguousarray(x.astype(np.float32, copy=False).T)  # [G, B]
    idx_maps = []
    I = np.asarray(I_i)
    for k in range(NCORES):
        Ik = I[k * GSH : (k + 1) * GSH]  # [256, 32, 8] values in [0, G)
        # call c gathers l=0..7 of group c for every partition p.
        # group c = gl*32 + s ; partition p holds g' = 2p + gl
        # list position j = i*128 + p  (i = l)
        Ikr = Ik.reshape(128, 2, S, L)  # [p, gl, s, l]
        lc = np.transpose(Ikr, (1, 2, 3, 0)).reshape(2 * S, L, 128)  # [c, i, p]
        flat = lc.reshape(NCALL, NIDX)  # j = i*128+p
        # wrapped: partition q slot t of call c holds flat[c, t*16 + q%16]
        w = flat.reshape(NCALL, NIDX // 16, 16)  # [c, t, q%16]
        w = np.transpose(w, (2, 0, 1)).reshape(16, NCALL * (NIDX // 16))
        idx = np.tile(w, (8, 1)).astype(np.int16)  # replicate to 128 partitions
        idx_maps.append(idx)
    return tbl, idx_maps


def kernel(x: np.ndarray, I_i: np.ndarray) -> np.ndarray:
    global _nc_cache, last_result
    if _nc_cache is None:
        _nc_cache = _build_nc()
    nc = _nc_cache
    tbl, idx_maps = _prep_inputs(x, I_i)
    in_maps = [{"tbl": tbl, "idx": idx_maps[k]} for k in range(NCORES)]
    res = run_bass_kernel_spmd(nc, in_maps, core_ids=list(range(NCORES)))
    last_result = res
    C = np.empty((B, G), dtype=np.float32)
    for k in range(NCORES):
        o = res.results[k]["c"].reshape(128, 2, B)  # [p, gl, b]
        C[:, k * GSH : (k + 1) * GSH] = np.transpose(o, (2, 0, 1)).reshape(B, GSH)
    return C


# revision 6
# speedup vs baseline: 3.4111x; 1.0157x over previous
"""Trainium2 kernel for nn_ClauseFunction (segment_reduce):
C[b,g] = softor_s(softand_l(x[b, I_i[g,s,l]])), gamma=1e-3.

Strategy: shard over G (each of 8 cores handles 256 g-columns; x replicated).
Per core: gather 256*32*8 = 65536 rows of xT (one row = x[:,j] for all 64 b,
256 bytes f32) from DRAM via gpsimd.dma_gather. Descriptor generation is the
bottleneck (Q7 SWDGE, ~8.5ns/idx per core pair), so gather calls are spread
over all 4 SWDGE queues: queue q runs on Q7 core pair (2q, 2q+1), so 4 pairs
generate descriptors concurrently (~2.2ns/idx effective).

With gamma=1e-3 the soft reductions are within ~1e-3 of hard min/max
(measured rel err 1.4e-3 << 2e-2 gate), so compute is plain min over l then
max over s on DVE. Per-chunk partial maxes keep the post-gather tail short,
and the last chunks use smaller gather calls so all 4 queues stay busy to
the end.

Layout: gathered tile [128 part, slots, 64 b]; partition p holds g' in
{2p, 2p+1}; group c = gl*32+s (gl=g' parity, s); a call covers whole groups
(8 l-slots each) for every partition.
"""

import numpy as np

import concourse.bacc as bacc
import concourse.bass as bass
import concourse.tile as tile
from concourse import library_config, mybir
from concourse.bass_utils import run_bass_kernel_spmd

B, G, S, L = 64, 2048, 32, 8
NCORES = 8
GSH = G // NCORES  # 256 g' per core
NQ = 4  # SWDGE queues (each on its own Q7 core pair)
GRP_PER_PART = GSH // 128 * S  # 64 groups (gl, s) per partition
# chunk schedule per gl-half: (idxs per call, calls per chunk); 4 calls per
# chunk, one per queue. A call of 1024 idxs covers 1 group (8 l-slots x 128
# partitions) and emits 64 descriptors/lane = the single_packet SDMA packet
# ceiling, so calls never exceed 1024 idxs. The per-chunk partial maxes keep
# the post-gather tail short without tapering.
HALF_SCHED = [(1024, 4)] * 8  # 32 groups
SCHED = HALF_SCHED * 2
NCHUNK = len(SCHED)

_nc_cache = None
last_result = None


def _v(t, dims, off=0):
    """View of tile t with explicit free-dim [stride, count] pairs (elements).

    Keeps the tile's own partition entry (stride = per-partition size)."""
    return bass.AP(tensor=t.tensor, offset=t.offset + off, ap=[list(t.ap[0])] + dims)


def _idx_cols():
    return sum(n * k for n, k in SCHED) // 16


def _build_nc():
    f32 = mybir.dt.float32
    nc = bacc.Bacc("TRN2", target_bir_lowering=False, num_swdge_queues=NQ)
    tbl_in = nc.dram_tensor("tbl", [G, B], f32, kind="ExternalInput")  # x.T
    idx_in = nc.dram_tensor("idx", [128, _idx_cols()], mybir.dt.int16, kind="ExternalInput")
    c_out = nc.dram_tensor("c", [128, 128], f32, kind="ExternalOutput")

    with tile.TileContext(nc) as tc:
        with (
            tc.tile_pool(name="singles", bufs=1) as singles,
            tc.tile_pool(name="gath", bufs=3) as gath,
            tc.tile_pool(name="small", bufs=2) as small,
        ):
            idxs = singles.tile([128, _idx_cols()], mybir.dt.int16)
            # split the idx load so the first chunk's gathers start early
            first_cols = SCHED[0][0] * SCHED[0][1] // 16
            nc.sync.dma_start(out=idxs[:, :first_cols], in_=idx_in[:, :first_cols])
            nc.sync.dma_start(out=idxs[:, first_cols:], in_=idx_in[:, first_cols:])
            vv = singles.tile([128, GRP_PER_PART, B], f32)  # per-group min_l
            pm = singles.tile([128, NCHUNK, B], f32)  # per-chunk partial max_s
            coff = 0  # idx column offset
            gbase = 0  # group offset
            call = 0
            for ch, (nidx, ncalls) in enumerate(SCHED):
                gpc = nidx // 1024  # groups per call
                cg = gpc * ncalls  # groups per chunk
                gt = gath.tile([128, cg * 8, B], f32, tag="gt")
                for ci in range(ncalls):
                    nc.gpsimd.dma_gather(
                        gt[:, ci * gpc * 8 : (ci + 1) * gpc * 8, :],
                        tbl_in[:, :],
                        idxs[:, coff : coff + nidx // 16],
                        num_idxs=nidx,
                        num_idxs_reg=nidx,
                        elem_size=B,
                        queue_num=call % NQ,
                    )
                    coff += nidx // 16
                    call += 1
                # gt slots = (grp cg, l 8), b innermost
                nc.vector.tensor_reduce(
                    out=vv[:, gbase : gbase + cg, :],
                    in_=_v(gt, [[8 * B, cg], [1, B], [B, 8]]),  # [grp, b, l]
                    axis=mybir.AxisListType.X,
                    op=mybir.AluOpType.min,
                )
                nc.vector.tensor_reduce(
                    out=pm[:, ch, :],
                    in_=_v(vv, [[1, B], [B, cg]], gbase * B),  # [b, grp]
                    axis=mybir.AxisListType.X,
                    op=mybir.AluOpType.max,
                )
                gbase += cg
                if gbase % 32 == 0:  # finished a gl half
                    gl = gbase // 32 - 1
                    nch = len(HALF_SCHED)
                    vm = small.tile([128, B], f32, tag="vm")
                    nc.vector.tensor_reduce(
                        out=vm,
                        in_=_v(pm, [[1, B], [B, nch]], gl * nch * B),  # [b, chunk]
                        axis=mybir.AxisListType.X,
                        op=mybir.AluOpType.max,
                    )
                    nc.sync.dma_start(out=c_out[:, gl * B : (gl + 1) * B], in_=vm)
    nc.finalize()
    return nc


def _prep_inputs(x: np.ndarray, I_i: np.ndarray):
    """Host-side layout: x transposed; per-core wrapped idx tensors."""
    tbl = np.ascontiguousarray(x.astype(np.float32, copy=False).T)  # [G, B]
    idx_maps = []
    I = np.asarray(I_i)
    for k in range(NCORES):
        Ik = I[k * GSH : (k + 1) * GSH]  # [256, 32, 8] values in [0, G)
        # group c = gl*32 + s ; partition p holds g' = 2p + gl
        Ikr = Ik.reshape(128, 2, S, L)  # [p, gl, s, l]
        lc = np.transpose(Ikr, (1, 2, 3, 0)).reshape(2 * S, L, 128)  # [c, l, p]
        parts = []
        gbase = 0
        for nidx, ncalls in SCHED:
            gpc = nidx // 1024
            for ci in range(ncalls):
                # call covers groups [gbase, gbase+gpc); list pos j = i*128+p,
                # i = gi*8 + l
                flat = lc[gbase : gbase + gpc].reshape(nidx)  # [gi, l, p] flat
                # wrapped: partition q slot t holds flat[t*16 + q%16]
                w = flat.reshape(nidx // 16, 16).T  # [16, nidx/16]
                parts.append(w)
                gbase += gpc
        wall = np.concatenate(parts, axis=1)  # [16, total_cols]
        idx = np.tile(wall, (8, 1)).astype(np.int16)  # replicate to 128 parts
        idx_maps.append(idx)
    return tbl, idx_maps


def kernel(x: np.ndarray, I_i: np.ndarray) -> np.ndarray:
    global _nc_cache, last_result
    if _nc_cache is None:
        _nc_cache = _build_nc()
    nc = _nc_cache
    tbl, idx_maps = _prep_inputs(x, I_i)
    in_maps = [{"tbl": tbl, "idx": idx_maps[k]} for k in range(NCORES)]
    res = run_bass_kernel_spmd(nc, in_maps, core_ids=list(range(NCORES)))
    last_result = res
    C = np.empty((B, G), dtype=np.float32)
    for k in range(NCORES):
        o = res.results[k]["c"].reshape(128, 2, B)  # [p, gl, b]
        C[:, k * GSH : (k + 1) * GSH] = np.transpose(o, (2, 0, 1)).reshape(B, GSH)
    return C
